# revision 21
# baseline (speedup 1.0000x reference)
"""Trainium2 Bass kernel for DetectPeaks (sliding-window NMS + top-2).

Computes, for xcorr [32, 3, 64, 8192] f32:
    x = |xcorr|
    smax = sliding max over time, window 301 (centered, clipped)
    scores = where(smax == x, x, 0)
    top2 values + indices along time  -> ([32,3,64,2] f32, [32,3,64,2] int32)

Strategy: flatten to 6144 independent rows, shard 768 rows per core across
8 cores (data parallel, no communication).  Per 128-row tile:
  - abs on the scalar engine (in place, in a 0.0-padded buffer)
  - van Herk / Gil-Werman sliding max at half resolution: per-150-block
    prefix/suffix max scans on h[v] = max(x[2v], x[2v+1])
    (tensor_tensor_scan with op=max on DVE), recombined per parity
  - masked scores via one fused custom-DVE select: m = (x >= smax) ? x : 0
  - top-2 extraction: peaks are >= 151 apart, so every 64-sample block of
    m has at most one nonzero.  Block "sums" (== the peak value, bit
    exactly) are computed on the otherwise-idle TENSOR engine: per
    128-wide chunk, PE-transpose -> PSUM, ScalarE copy -> SBUF, then a
    selector matmul accumulates the two 64-blocks of the chunk into a
    PSUM S^T[128 blocks, 128 rows] tile.  After a final transpose,
    max8 over the 128 block maxes gives the top-8 peak values and one
    find_index8 over the full-res masked row gives exact indices.
    The max8/find_index8 tail of tile i is emitted after tile i+1's
    scan chain so the in-order DVE stream never waits on the PE chain.
"""

import numpy as np

NB, NC, NX, NT = 32, 3, 64, 8192
KERNEL = 301
PAD = KERNEL // 2  # 150
B = KERNEL  # van Herk block size
NBLK = 29  # ceil((PAD + NT + PAD) / B) -> cover xp coords [0, 8491]
LPAD = NBLK * B  # 8729
N_CORES = 8
ROWS = NB * NC * NX  # 6144
ROWS_PER_CORE = ROWS // N_CORES  # 768
P_DIM = 128
NTILE = ROWS_PER_CORE // P_DIM  # 6
BMC = 64  # block size for the block-max top-k (64 < 151 => <=1 peak/block)
NBM = NT // BMC  # 128 block maxes per row
NCHUNK = NT // P_DIM  # 64 transpose chunks per tile
import os as _os

PRE_LEVELS = int(_os.environ.get("KERNEL_PRE_LEVELS", "2"))

_cached = None
_select_ge = None


def _register_select_ge():
    """Register a fused custom-DVE op  out = (in0 >= in1) ? in0 : 0  at
    runtime (the package's dve_ops.py is read-only; the documented way to
    add an op is appending to its OPS registry)."""
    global _select_ge
    if _select_ge is not None:
        return _select_ge
    import concourse.dve_ops as dve_ops_mod
    from concourse.dve_spec import Spec, Src0, Src1, Zero, select, lower
    from concourse.dve_spec import _has_src1
    from concourse.dve_uop import DveOpSpec

    name = "SELECT_GE_PEAK_ANT"
    for op in dve_ops_mod.OPS:
        if op.name == name:
            _select_ge = op
            return op

    spec = Spec(
        body=select(Src0 >= Src1, Src0, Zero),
        reference=lambda in0, in1, s0, s1, imm2: np.where(
            in0 >= in1, in0, 0.0
        ).astype(np.float32),
    )
    row = dve_ops_mod._CUSTOM_DVE_ROW_BASE + len(dve_ops_mod.OPS)
    assert row < 0x20
    shas = {}
    for ver in ("v3", "v4"):
        s = DveOpSpec(
            name=name, opcode=row, uops=lower(spec, ver=ver), rd1_en=_has_src1(spec)
        )
        shas[ver] = s.sha(ver)
    op = dve_ops_mod.DveOp(name, spec, subdim=False, uops_sha=shas)
    dve_ops_mod.OPS.append(op)
    dve_ops_mod._SUB_OPCODE_FOR_NAME[name] = row
    dve_ops_mod.CUSTOM_DVE_SPECS[name] = spec
    _select_ge = op
    return op


def _build(rows_per_core=ROWS_PER_CORE):
    import concourse.mybir as mybir
    from concourse.bacc import Bacc
    from concourse.tile import TileContext
    from concourse.masks import make_identity

    f32 = mybir.dt.float32
    Alu = mybir.AluOpType
    Act = mybir.ActivationFunctionType
    n_tiles = rows_per_core // P_DIM
    sel_op = _register_select_ge()

    # Bacc (not plain Bass): its finalize() runs generate_event_semaphores,
    # which splits multi-sem waits into EventSemaphore prefixes — TRN2
    # instructions only have a single wait slot.
    nc = Bacc(None, target_bir_lowering=False)
    x_in = nc.dram_tensor("x", [rows_per_core, NT], f32, kind="ExternalInput")
    out_vals = nc.dram_tensor("out_vals", [rows_per_core, 8], f32, kind="ExternalOutput")
    out_idx = nc.dram_tensor("out_idx", [rows_per_core, 8], mybir.dt.uint32, kind="ExternalOutput")

    # Half-resolution (parity) van Herk: the expensive segmented scans run
    # on h[v] = max(x[2v], x[2v+1]) with window 150 / block 150, then the
    # full-res sliding max is reassembled per parity:
    #   smax[2u]   = max(H150[u],   xp[2u+300])
    #   smax[2u+1] = max(xp[2u+1],  H150[u+1])
    # with H150[v] = max(h[v..v+149]) = max(Sh[v], Ph[v+149]).
    LP2 = LPAD + 1          # 8730, even
    HLEN = LP2 // 2         # 4365
    B2 = 150
    HPAD = 30 * B2          # 4500
    MH = NT // 2 + 1        # 4097 H150 values needed
    PHE = B2 - 1 + MH       # 4246, Ph read range
    SHE = (NT // 2 // B2) * B2 + B2 - 1  # 4199, end of Sh's block

    with TileContext(nc) as tc:
        with (
            tc.tile_pool(name="const", bufs=1) as cpool,
            tc.tile_pool(name="big", bufs=2) as bigpool,
            tc.tile_pool(name="scan", bufs=1) as scanpool,
            tc.tile_pool(name="sc", bufs=2) as scpool,
            tc.tile_pool(name="small", bufs=2) as smallpool,
            tc.tile_pool(name="stage", bufs=3) as stagepool,
            tc.tile_pool(name="ps", bufs=3, space="PSUM") as pspool,
            tc.tile_pool(name="psS", bufs=1, space="PSUM") as psSpool,
            tc.tile_pool(name="ps2", bufs=2, space="PSUM") as ps2pool,
        ):
            # Segment mask for block-restarting max scans over h: zeros at
            # multiples of 150 (scan state = max(G2[v]*state, h[v]) restarts
            # at every 0 since all data >= 0). G2[1:] reversed provides the
            # restart markers for the reversed (suffix) scan.
            G2 = cpool.tile([P_DIM, HPAD + 1], f32, tag="G2")
            nc.gpsimd.memset(G2[:, :], 1.0)
            nc.gpsimd.memset(G2[:, 0:HPAD + 1:B2], 0.0)
            ident = cpool.tile([P_DIM, P_DIM], f32, tag="ident")
            make_identity(nc, ident[:, :])
            # Wbig[p, k] = 1 iff k == WOFF + p//blk; the slice
            # Wbig[:, WOFF-per*c : WOFF-per*c+128] is the chunk-c selector
            # placing its `per` blocks at rows per*c .. per*c+per-1 of the
            # accumulated S^T.
            blk0 = BMC >> PRE_LEVELS
            per0 = P_DIM // blk0
            WOFF0 = P_DIM - per0
            Wbig = cpool.tile([P_DIM, 2 * P_DIM], f32, tag="W")
            nc.gpsimd.memset(Wbig[:, :], 0.0)
            for q in range(per0):
                nc.gpsimd.memset(
                    Wbig[q * blk0:(q + 1) * blk0, WOFF0 + q:WOFF0 + q + 1], 1.0
                )

            prev = None  # deferred tail state: (rows, m, Ssb)
            for i in range(n_tiles):
                rows = slice(i * P_DIM, (i + 1) * P_DIM)
                xp = bigpool.tile([P_DIM, LP2], f32, tag="xp")
                interior = xp[:, PAD:PAD + NT]
                # Pads + abs all on the scalar engine (|0|=0 keeps pads valid);
                # pads only matter as neutral (<= data) elements.  Tile 0 is
                # fully on the critical path (nothing to overlap with), so
                # chunk its DMA+abs to let compute start sooner.
                nchunk = 4 if i == 0 else 1
                CH = NT // nchunk
                for c in range(nchunk):
                    sl = slice(PAD + c * CH, PAD + (c + 1) * CH)
                    nc.sync.dma_start(xp[:, sl], x_in[rows, c * CH:(c + 1) * CH])
                    nc.scalar.activation(xp[:, sl], xp[:, sl], Act.Abs)
                nc.scalar.memzero(xp[:, 0:PAD])
                nc.scalar.memzero(xp[:, PAD + NT:LP2])

                h = scanpool.tile([P_DIM, HLEN], f32, tag="h")
                nc.vector.tensor_tensor(
                    out=h[:, :], in0=xp[:, 0:LP2:2], in1=xp[:, 1:LP2:2],
                    op=Alu.max,
                )

                # Trimmed scan ranges: Ph is only read on [149, 4246) and Sh
                # on [0, 4097) (all within real h data, so no tail memset).
                # The suffix scan runs IN PLACE over h (h is dead after the
                # two scans; each element is read once then overwritten).
                Ph = scanpool.tile([P_DIM, PHE], f32, tag="Ph")
                nc.vector.tensor_tensor_scan(
                    Ph[:, 0:PHE], G2[:, 0:PHE], h[:, 0:PHE], 0.0,
                    op0=Alu.mult, op1=Alu.max,
                )
                nc.vector.tensor_tensor_scan(
                    h[:, SHE::-1], G2[:, 1:SHE + 2][:, ::-1], h[:, SHE::-1], 0.0,
                    op0=Alu.mult, op1=Alu.max,
                )

                # H150[v] = max(Sh[v], Ph[v+149]), v in [0, 4097)
                mh = scanpool.tile([P_DIM, MH], f32, tag="mh")
                nc.vector.tensor_tensor(
                    out=mh[:, :], in0=h[:, 0:MH], in1=Ph[:, B2 - 1:B2 - 1 + MH],
                    op=Alu.max,
                )
                # reassemble full-res smax into m (even/odd interleaved)
                m = scpool.tile([P_DIM, NT], f32, tag="m")
                nc.vector.tensor_tensor(
                    out=m[:, 0:NT:2], in0=mh[:, 0:NT // 2],
                    in1=xp[:, 2 * PAD:2 * PAD + NT:2], op=Alu.max,
                )
                nc.vector.tensor_tensor(
                    out=m[:, 1:NT:2], in0=xp[:, 1:NT:2], in1=mh[:, 1:NT // 2 + 1],
                    op=Alu.max,
                )
                # m <- (x >= smax) ? x : 0 in ONE fused custom-DVE pass
                # (replaces the is_ge + mult pair).
                nc.vector._custom_dve(sel_op, out=m[:, :], in0=interior, in1=m[:, :])

                # ---- block-max pipeline ----
                # A few cheap adjacent-pairwise-max levels on DVE shrink the
                # data fed to the Tensor-engine transpose+matmul chain (whose
                # cost is dominated by per-instruction overhead, so halving
                # the chunk count nearly halves it).  Pairwise MAX cannot
                # double tied values, so ghosts only arise at the PE sum
                # stage (filtered in run()).
                red = m[:, :]
                rw = NT
                for lv in range(PRE_LEVELS):
                    rw //= 2
                    # Scratch: reuse mh's storage (dead once re/ro ran; the
                    # next tile's mh-combine rewrite lands ~40us later, well
                    # after the short PE chain drains it).  The lv>0 folds
                    # are in-place pair folds (write index trails both read
                    # indices, so streaming is safe).
                    nc.vector.tensor_tensor(
                        out=mh[:, 0:rw], in0=red[:, 0:2 * rw:2],
                        in1=red[:, 1:2 * rw:2], op=Alu.max,
                    )
                    red = mh[:, 0:rw]
                nchunks = rw // P_DIM
                blk = BMC >> PRE_LEVELS  # block width at reduced resolution
                per = P_DIM // blk       # blocks per chunk
                WOFF = P_DIM - per
                ST_ps = ps2pool.tile([P_DIM, P_DIM], f32, tag="ST")
                for c in range(nchunks):
                    tp = pspool.tile([P_DIM, P_DIM], f32, tag="tp")
                    nc.tensor.transpose(
                        tp[:, :], red[:, c * P_DIM:(c + 1) * P_DIM], ident[:, :]
                    )
                    ts = stagepool.tile([P_DIM, P_DIM], f32, tag="ts")
                    nc.scalar.activation(ts[:, :], tp[:, :], Act.Copy)
                    nc.tensor.matmul(
                        ST_ps[:, :],
                        Wbig[:, WOFF - per * c:WOFF - per * c + P_DIM],
                        ts[:, :],
                        start=(c == 0), stop=(c == nchunks - 1),
                    )
                STs = stagepool.tile([P_DIM, P_DIM], f32, tag="STs")
                nc.scalar.activation(STs[:, :], ST_ps[:, :], Act.Copy)
                S_ps = psSpool.tile([P_DIM, P_DIM], f32, tag="S")
                nc.tensor.transpose(S_ps[:, :], STs[:, :], ident[:, :])
                Ssb = smallpool.tile([P_DIM, NBM], f32, tag="Ssb")
                nc.scalar.activation(Ssb[:, :], S_ps[:, :], Act.Copy)

                # ---- deferred DVE tail of the PREVIOUS tile ----
                if prev is not None:
                    _emit_tail(nc, out_vals, out_idx, smallpool, mybir, *prev)
                prev = (rows, m, Ssb)
            _emit_tail(nc, out_vals, out_idx, smallpool, mybir, *prev)
    return nc


def _emit_tail(nc, out_vals, out_idx, smallpool, mybir, rows, m, Ssb):
    """Top-8 of the 128 block maxes + exact indices from the masked row."""
    v8 = smallpool.tile([P_DIM, 8], mybir.dt.float32, tag="v8")
    i8 = smallpool.tile([P_DIM, 8], mybir.dt.uint32, tag="i8")
    nc.vector.max(out=v8, in_=Ssb[:, :])
    nc.vector.max_index(out=i8, in_max=v8, in_values=m[:, :])
    nc.sync.dma_start(out_vals[rows, :], v8)
    nc.sync.dma_start(out_idx[rows, :], i8)


def _get_module():
    global _cached
    if _cached is None:
        _cached = _build()
        # run_bass_via_pjrt serializes the module as-is; Bacc.finalize()
        # runs register allocation + event-semaphore legalization.
        _cached.finalize()
    return _cached


def run(xcorr: np.ndarray, trace: bool = False, **spmd_kwargs):
    from concourse.bass_utils import run_bass_kernel_spmd

    x = np.ascontiguousarray(np.asarray(xcorr, dtype=np.float32).reshape(ROWS, NT))
    nc = _get_module()
    in_maps = [
        {"x": x[c * ROWS_PER_CORE:(c + 1) * ROWS_PER_CORE]} for c in range(N_CORES)
    ]
    res = run_bass_kernel_spmd(
        nc, in_maps, core_ids=list(range(N_CORES)), trace=trace, **spmd_kwargs
    )
    vals8 = np.concatenate([r["out_vals"] for r in res.results], axis=0)
    idx8 = np.concatenate([r["out_idx"] for r in res.results], axis=0)
    # Exact-duplicate values inside one 64-block (f32 birthday collisions in
    # the input) make the PE block-sum produce a "ghost" doubled value whose
    # find_index8 lookup misses (sentinel 0xFFFFFFFF).  Drop ghosts and keep
    # the first two valid candidates per row.
    ghost = idx8 == np.uint32(0xFFFFFFFF)
    if ghost.any():
        order = np.argsort(ghost, axis=1, kind="stable")[:, :2]
        vals = np.take_along_axis(vals8, order, 1)
        idx = np.take_along_axis(idx8, order, 1)
    else:
        vals = vals8[:, :2]
        idx = idx8[:, :2]
    topk_score = vals.reshape(NB, NC, NX, 2).astype(np.float32)
    topk_idx = idx.reshape(NB, NC, NX, 2).astype(np.int32)
    return (topk_score, topk_idx), res


def kernel(xcorr: np.ndarray, nlag=None, **_unused):
    out, _ = run(xcorr)
    return out


# revision 25
# speedup vs baseline: 1.1485x; 1.1485x over previous
"""Trainium2 Bass kernel for DetectPeaks (sliding-window NMS + top-2).

Computes, for xcorr [32, 3, 64, 8192] f32:
    x = |xcorr|
    smax = sliding max over time, window 301 (centered, clipped)
    scores = where(smax == x, x, 0)
    top2 values + indices along time  -> ([32,3,64,2] f32, [32,3,64,2] int32)

Strategy: flatten to 6144 independent rows, shard 768 rows per core across
8 cores (data parallel, no communication).  Per 128-row tile:
  - abs on the scalar engine (in place, in a 0.0-padded buffer)
  - van Herk / Gil-Werman sliding max at half resolution: per-150-block
    prefix/suffix max scans on h[v] = max(x[2v], x[2v+1])
    (tensor_tensor_scan with op=max on DVE), recombined per parity
  - masked scores via one fused custom-DVE select: m = (x >= smax) ? x : 0
  - top-2 extraction: peaks are >= 151 apart, so every 64-sample block of
    m has at most one nonzero.  Block "sums" (== the peak value, bit
    exactly) are computed on the otherwise-idle TENSOR engine: per
    128-wide chunk, PE-transpose -> PSUM, ScalarE copy -> SBUF, then a
    selector matmul accumulates the two 64-blocks of the chunk into a
    PSUM S^T[128 blocks, 128 rows] tile.  After a final transpose,
    max8 over the 128 block maxes gives the top-8 peak values and one
    find_index8 over the full-res masked row gives exact indices.
    The max8/find_index8 tail of tile i is emitted after tile i+1's
    scan chain so the in-order DVE stream never waits on the PE chain.
"""

import numpy as np

NB, NC, NX, NT = 32, 3, 64, 8192
KERNEL = 301
PAD = KERNEL // 2  # 150
B = KERNEL  # van Herk block size
NBLK = 29  # ceil((PAD + NT + PAD) / B) -> cover xp coords [0, 8491]
LPAD = NBLK * B  # 8729
N_CORES = 8
ROWS = NB * NC * NX  # 6144
ROWS_PER_CORE = ROWS // N_CORES  # 768
P_DIM = 128
NTILE = ROWS_PER_CORE // P_DIM  # 6
BMC = 64  # block size for the block-max top-k (64 < 151 => <=1 peak/block)
NBM = NT // BMC  # 128 block maxes per row
NCHUNK = NT // P_DIM  # 64 transpose chunks per tile
import os as _os

PRE_LEVELS = int(_os.environ.get("KERNEL_PRE_LEVELS", "2"))

_cached = None
_select_ge = None


def _register_select_ge():
    """Register a fused custom-DVE op  out = (in0 >= in1) ? in0 : 0  at
    runtime (the package's dve_ops.py is read-only; the documented way to
    add an op is appending to its OPS registry)."""
    global _select_ge
    if _select_ge is not None:
        return _select_ge
    import concourse.dve_ops as dve_ops_mod
    from concourse.dve_spec import Spec, Src0, Src1, Zero, select, lower
    from concourse.dve_spec import _has_src1
    from concourse.dve_uop import DveOpSpec

    name = "SELECT_GE_PEAK_ANT"
    for op in dve_ops_mod.OPS:
        if op.name == name:
            _select_ge = op
            return op

    spec = Spec(
        body=select(Src0 >= Src1, Src0, Zero),
        reference=lambda in0, in1, s0, s1, imm2: np.where(
            in0 >= in1, in0, 0.0
        ).astype(np.float32),
    )
    row = dve_ops_mod._CUSTOM_DVE_ROW_BASE + len(dve_ops_mod.OPS)
    assert row < 0x20
    shas = {}
    for ver in ("v3", "v4"):
        s = DveOpSpec(
            name=name, opcode=row, uops=lower(spec, ver=ver), rd1_en=_has_src1(spec)
        )
        shas[ver] = s.sha(ver)
    op = dve_ops_mod.DveOp(name, spec, subdim=False, uops_sha=shas)
    dve_ops_mod.OPS.append(op)
    dve_ops_mod._SUB_OPCODE_FOR_NAME[name] = row
    dve_ops_mod.CUSTOM_DVE_SPECS[name] = spec
    _select_ge = op
    return op


def _build(rows_per_core=ROWS_PER_CORE):
    import concourse.mybir as mybir
    from concourse.bacc import Bacc
    from concourse.tile import TileContext
    from concourse.masks import make_identity

    f32 = mybir.dt.float32
    Alu = mybir.AluOpType
    Act = mybir.ActivationFunctionType
    n_tiles = rows_per_core // P_DIM
    sel_op = _register_select_ge()

    # Bacc (not plain Bass): its finalize() runs generate_event_semaphores,
    # which splits multi-sem waits into EventSemaphore prefixes — TRN2
    # instructions only have a single wait slot.
    nc = Bacc(None, target_bir_lowering=False)
    x_in = nc.dram_tensor("x", [rows_per_core, NT], f32, kind="ExternalInput")
    out_vals = nc.dram_tensor("out_vals", [rows_per_core, 8], f32, kind="ExternalOutput")
    out_idx = nc.dram_tensor("out_idx", [rows_per_core, 8], mybir.dt.uint32, kind="ExternalOutput")

    # Half-resolution (parity) van Herk: the expensive segmented scans run
    # on h[v] = max(x[2v], x[2v+1]) with window 150 / block 150, then the
    # full-res sliding max is reassembled per parity:
    #   smax[2u]   = max(H150[u],   xp[2u+300])
    #   smax[2u+1] = max(xp[2u+1],  H150[u+1])
    # with H150[v] = max(h[v..v+149]) = max(Sh[v], Ph[v+149]).
    B2 = 150
    MH = NT // 2 + 1        # 4097 H150 values needed
    PHE = B2 - 1 + MH       # 4246, Ph read range
    SHE = (NT // 2 // B2) * B2 + B2 - 1  # 4199, end of Sh's block
    HLEN = PHE + 1          # 4247 h values suffice for both scans
    LP2 = 2 * HLEN          # 8494 xp values (>= PAD + NT + PAD = 8492)
    GLEN = PHE + 1          # G2 length

    with TileContext(nc) as tc:
        with (
            tc.tile_pool(name="const", bufs=1) as cpool,
            tc.tile_pool(name="big", bufs=2) as bigpool,
            tc.tile_pool(name="scan", bufs=1) as scanpool,
            tc.tile_pool(name="sc", bufs=2) as scpool,
            tc.tile_pool(name="small", bufs=2) as smallpool,
            tc.tile_pool(name="stage", bufs=2) as stagepool,
            tc.tile_pool(name="red", bufs=2) as redpool,
            tc.tile_pool(name="ps", bufs=3, space="PSUM") as pspool,
            tc.tile_pool(name="psS", bufs=1, space="PSUM") as psSpool,
            tc.tile_pool(name="ps2", bufs=2, space="PSUM") as ps2pool,
        ):
            # Segment mask for block-restarting max scans over h: zeros at
            # multiples of 150 (scan state = max(G2[v]*state, h[v]) restarts
            # at every 0 since all data >= 0). G2[1:] reversed provides the
            # restart markers for the reversed (suffix) scan.
            # G2 in bf16 (0.0/1.0 are exact; scan state is fp32 internally)
            G2 = cpool.tile([P_DIM, GLEN], mybir.dt.bfloat16, tag="G2")
            nc.gpsimd.memset(G2[:, :], 1.0)
            nc.gpsimd.memset(G2[:, 0:GLEN:B2], 0.0)
            ident = cpool.tile([P_DIM, P_DIM], f32, tag="ident")
            make_identity(nc, ident[:, :])
            # Wbig[p, k] = 1 iff k == WOFF + p//blk; the slice
            # Wbig[:, WOFF-per*c : WOFF-per*c+128] is the chunk-c selector
            # placing its `per` blocks at rows per*c .. per*c+per-1 of the
            # accumulated S^T.
            blk0 = BMC >> PRE_LEVELS
            per0 = P_DIM // blk0
            WOFF0 = P_DIM - per0
            Wbig = cpool.tile([P_DIM, 2 * P_DIM], f32, tag="W")
            nc.gpsimd.memset(Wbig[:, :], 0.0)
            for q in range(per0):
                nc.gpsimd.memset(
                    Wbig[q * blk0:(q + 1) * blk0, WOFF0 + q:WOFF0 + q + 1], 1.0
                )

            prev = None  # deferred tail state: (rows, m, Ssb)
            for i in range(n_tiles):
                rows = slice(i * P_DIM, (i + 1) * P_DIM)
                xp = bigpool.tile([P_DIM, LP2], f32, tag="xp")
                interior = xp[:, PAD:PAD + NT]
                # Pads + abs all on the scalar engine (|0|=0 keeps pads valid);
                # pads only matter as neutral (<= data) elements.  Tile 0 is
                # fully on the critical path (nothing to overlap with), so
                # chunk its DMA+abs to let compute start sooner.
                nchunk = 4 if i == 0 else 1
                CH = NT // nchunk
                for c in range(nchunk):
                    sl = slice(PAD + c * CH, PAD + (c + 1) * CH)
                    nc.sync.dma_start(xp[:, sl], x_in[rows, c * CH:(c + 1) * CH])
                    nc.scalar.activation(xp[:, sl], xp[:, sl], Act.Abs)
                nc.scalar.memzero(xp[:, 0:PAD])
                nc.scalar.memzero(xp[:, PAD + NT:LP2])

                h = scanpool.tile([P_DIM, HLEN], f32, tag="h")
                nc.vector.tensor_tensor(
                    out=h[:, :], in0=xp[:, 0:LP2:2], in1=xp[:, 1:LP2:2],
                    op=Alu.max,
                )

                # Trimmed scan ranges: Ph is only read on [149, 4246) and Sh
                # on [0, 4097) (all within real h data, so no tail memset).
                # The suffix scan runs IN PLACE over h (h is dead after the
                # two scans; each element is read once then overwritten).
                Ph = scanpool.tile([P_DIM, PHE], f32, tag="Ph")
                nc.vector.tensor_tensor_scan(
                    Ph[:, 0:PHE], G2[:, 0:PHE], h[:, 0:PHE], 0.0,
                    op0=Alu.mult, op1=Alu.max,
                )
                nc.vector.tensor_tensor_scan(
                    h[:, SHE::-1], G2[:, 1:SHE + 2][:, ::-1], h[:, SHE::-1], 0.0,
                    op0=Alu.mult, op1=Alu.max,
                )

                # H150[v] = max(Sh[v], Ph[v+149]), v in [0, 4097)
                mh = scanpool.tile([P_DIM, MH], f32, tag="mh")
                nc.vector.tensor_tensor(
                    out=mh[:, :], in0=h[:, 0:MH], in1=Ph[:, B2 - 1:B2 - 1 + MH],
                    op=Alu.max,
                )
                # reassemble full-res smax into m (even/odd interleaved)
                m = scpool.tile([P_DIM, NT], f32, tag="m")
                nc.vector.tensor_tensor(
                    out=m[:, 0:NT:2], in0=mh[:, 0:NT // 2],
                    in1=xp[:, 2 * PAD:2 * PAD + NT:2], op=Alu.max,
                )
                nc.vector.tensor_tensor(
                    out=m[:, 1:NT:2], in0=xp[:, 1:NT:2], in1=mh[:, 1:NT // 2 + 1],
                    op=Alu.max,
                )
                # m <- (x >= smax) ? x : 0 in ONE fused custom-DVE pass
                # (replaces the is_ge + mult pair).
                nc.vector._custom_dve(sel_op, out=m[:, :], in0=interior, in1=m[:, :])

                # ---- block-max pipeline ----
                # A few cheap adjacent-pairwise-max levels on DVE shrink the
                # data fed to the Tensor-engine transpose+matmul chain (whose
                # cost is dominated by per-instruction overhead, so halving
                # the chunk count nearly halves it).  Pairwise MAX cannot
                # double tied values, so ghosts only arise at the PE sum
                # stage (filtered in run()).
                red = m[:, :]
                rw = NT
                if PRE_LEVELS:
                    # L1 into mh's storage (dead once re/ro ran; consumed
                    # immediately in-order by L2, so no cross-tile hazard),
                    # L2 into a dedicated double-buffered tile that the PE
                    # chain reads without blocking the next tile's DVE work.
                    assert PRE_LEVELS == 2
                    nc.vector.tensor_tensor(
                        out=mh[:, 0:NT // 2], in0=red[:, 0:NT:2],
                        in1=red[:, 1:NT:2], op=Alu.max,
                    )
                    red2 = redpool.tile([P_DIM, NT // 4], f32, tag="red2")
                    nc.vector.tensor_tensor(
                        out=red2[:, :], in0=mh[:, 0:NT // 2:2],
                        in1=mh[:, 1:NT // 2:2], op=Alu.max,
                    )
                    red = red2[:, :]
                    rw = NT // 4
                nchunks = rw // P_DIM
                blk = BMC >> PRE_LEVELS  # block width at reduced resolution
                per = P_DIM // blk       # blocks per chunk
                WOFF = P_DIM - per
                ST_ps = ps2pool.tile([P_DIM, P_DIM], f32, tag="ST")
                for c in range(nchunks):
                    tp = pspool.tile([P_DIM, P_DIM], f32, tag="tp")
                    nc.tensor.transpose(
                        tp[:, :], red[:, c * P_DIM:(c + 1) * P_DIM], ident[:, :]
                    )
                    ts = stagepool.tile([P_DIM, P_DIM], f32, tag="ts")
                    nc.scalar.activation(ts[:, :], tp[:, :], Act.Copy)
                    nc.tensor.matmul(
                        ST_ps[:, :],
                        Wbig[:, WOFF - per * c:WOFF - per * c + P_DIM],
                        ts[:, :],
                        start=(c == 0), stop=(c == nchunks - 1),
                    )
                STs = stagepool.tile([P_DIM, P_DIM], f32, tag="STs")
                nc.scalar.activation(STs[:, :], ST_ps[:, :], Act.Copy)
                S_ps = psSpool.tile([P_DIM, P_DIM], f32, tag="S")
                nc.tensor.transpose(S_ps[:, :], STs[:, :], ident[:, :])
                Ssb = smallpool.tile([P_DIM, NBM], f32, tag="Ssb")
                nc.scalar.activation(Ssb[:, :], S_ps[:, :], Act.Copy)

                # ---- deferred DVE tail of the PREVIOUS tile ----
                if prev is not None:
                    _emit_tail(nc, out_vals, out_idx, smallpool, mybir, *prev)
                prev = (rows, m, Ssb)
            _emit_tail(nc, out_vals, out_idx, smallpool, mybir, *prev)
    return nc


def _emit_tail(nc, out_vals, out_idx, smallpool, mybir, rows, m, Ssb):
    """Top-8 of the 128 block maxes + exact indices from the masked row."""
    v8 = smallpool.tile([P_DIM, 8], mybir.dt.float32, tag="v8")
    i8 = smallpool.tile([P_DIM, 8], mybir.dt.uint32, tag="i8")
    nc.vector.max(out=v8, in_=Ssb[:, :])
    nc.vector.max_index(out=i8, in_max=v8, in_values=m[:, :])
    nc.sync.dma_start(out_vals[rows, :], v8)
    nc.sync.dma_start(out_idx[rows, :], i8)


def _get_module():
    global _cached
    if _cached is None:
        _cached = _build()
        # run_bass_via_pjrt serializes the module as-is; Bacc.finalize()
        # runs register allocation + event-semaphore legalization.
        _cached.finalize()
    return _cached


def run(xcorr: np.ndarray, trace: bool = False, **spmd_kwargs):
    from concourse.bass_utils import run_bass_kernel_spmd

    x = np.ascontiguousarray(np.asarray(xcorr, dtype=np.float32).reshape(ROWS, NT))
    nc = _get_module()
    in_maps = [
        {"x": x[c * ROWS_PER_CORE:(c + 1) * ROWS_PER_CORE]} for c in range(N_CORES)
    ]
    res = run_bass_kernel_spmd(
        nc, in_maps, core_ids=list(range(N_CORES)), trace=trace, **spmd_kwargs
    )
    vals8 = np.concatenate([r["out_vals"] for r in res.results], axis=0)
    idx8 = np.concatenate([r["out_idx"] for r in res.results], axis=0)
    # Exact-duplicate values inside one 64-block (f32 birthday collisions in
    # the input) make the PE block-sum produce a "ghost" doubled value whose
    # find_index8 lookup misses (sentinel 0xFFFFFFFF).  Drop ghosts and keep
    # the first two valid candidates per row.
    ghost = idx8 == np.uint32(0xFFFFFFFF)
    if ghost.any():
        order = np.argsort(ghost, axis=1, kind="stable")[:, :2]
        vals = np.take_along_axis(vals8, order, 1)
        idx = np.take_along_axis(idx8, order, 1)
    else:
        vals = vals8[:, :2]
        idx = idx8[:, :2]
    topk_score = vals.reshape(NB, NC, NX, 2).astype(np.float32)
    topk_idx = idx.reshape(NB, NC, NX, 2).astype(np.int32)
    return (topk_score, topk_idx), res


def kernel(xcorr: np.ndarray, nlag=None, **_unused):
    out, _ = run(xcorr)
    return out
